# revision 21
# baseline (speedup 1.0000x reference)
"""MoE routing kernel for Trainium2, 8 NeuronCores.

Strategy (expert-parallel, mixed precision, one device launch):
  Host: gating softmax + top-k in float64 (selection is exact vs the
  f32 reference since top-k margins dwarf f32 rounding noise). Per
  expert, sort its assigned tokens by gate value p (descending):
    - top CB=512 (large p)  -> bf16 path (gate-pre-scaled tokens)
    - next C8=512 (small p) -> fp8 e4m3 path (gate-pre-scaled), run
      with MatmulPerfMode.DoubleRow: 2x PE throughput. The fp8
      quantization error lands only on the low-gate half of the
      assignments, keeping total L2 error ~1.6e-2 (< 2e-2 gate).
    - remainder (~2%)       -> host f32 (standard capacity overflow)
  All tensors are packed on the host directly into SBUF layout
  [P, KT, free] so DMA descriptors have multi-KB contiguous runs per
  partition. All DMA queues share ~320 GB/s aggregate, so loads are
  emitted in compute-phase order, round-robin across the 3 queues
  (sync/scalar HWDGE + gpsimd SWDGE). The fp8 copies of the expert
  weights are produced ON DEVICE (DVE bf16->fp8 cast) instead of
  being loaded, cutting input traffic 9MB -> 7MB per core.
  Device phase order bf16(e0), fp8(e0), bf16(e1), fp8(e1): the long
  bf16 phases lead, giving the DMA stream runway; fp8 phases then
  only need the small x8 tensors plus the already-resident weights.
  fp32 PSUM; bf16 outputs (halves store traffic); warmup matmuls
  cover the PE clock ramp; small final waves keep the drain short.

PE floor: per core 128 bf16 + 64 DR matmul instrs at ~216ns issue
rate ~ 41.5us. Measured baseline (all-bf16) was ~76-81us.
"""
import numpy as np
from contextlib import ExitStack

import ml_dtypes

import concourse.mybir as mybir
from concourse import bacc, tile
from concourse.bass_utils import run_bass_kernel_spmd

NCORES = 8
P = 128
F32 = mybir.dt.float32
BF16 = mybir.dt.bfloat16
F8 = mybir.dt.float8e4
NPBF16 = ml_dtypes.bfloat16
NPF8 = ml_dtypes.float8_e4m3

# device-side bf16->fp8 weight cast (saves 2MB/core of input DMA).
# Set False to load host-quantized fp8 weights instead.
DEVICE_CAST_W8 = True

# test-harness knobs (ignored in normal use)
TRACE = False
LAST_EXEC_NS = []
LAST_RESULTS = {}

_cache = {}


def _warmup_pe(nc, pool, ps_pool, n_mm, tag="ps"):
    """Dummy bf16 matmuls on scratch data, issued at kernel start so the
    PE's HAM clock-gate ramps toward 2.4 GHz while the input DMAs
    stream in."""
    wt = pool.tile([P, 512], BF16, name="warm_sb")
    nc.vector.memset(wt[:], 1.0)
    wp = ps_pool.tile([P, 512], F32, name="warm_ps", tag=tag)
    for _ in range(n_mm):
        nc.tensor.matmul(wp[:], wt[:, :P], wt[:], start=True, stop=True)
    return wt, wp


def _build_mixed(CB, C8, DIN, DOUT, EPC):
    """Per-core expert compute, mixed bf16/fp8-DoubleRow.

    Inputs : xbT [EPC, P, KT, CB]   bf16 (pre-scaled tokens, SBUF layout)
             x8T [EPC, P, KT, C8]   f8e4 (pre-scaled tokens, SBUF layout)
             wb  [EPC, P, KT, DOUT] bf16
             (w8 [EPC, P, KT, DOUT] f8e4 -- only if not DEVICE_CAST_W8)
    Output : yout [EPC, 2, MT, P, DOUT] bf16  (path 0 = bf16, 1 = fp8)
    """
    key = ("mix", CB, C8, DIN, DOUT, EPC, DEVICE_CAST_W8)
    if key in _cache:
        return _cache[key]
    KT = DIN // P
    MTB = CB // P
    MT8 = C8 // P
    NF = 512
    assert DOUT % NF == 0 and KT % 2 == 0
    NT = DOUT // NF
    assert EPC == 2
    nc = bacc.Bacc("TRN2", target_bir_lowering=False, debug=False,
                   num_devices=NCORES)
    xbT = nc.dram_tensor("xbT", [EPC, P, KT, CB], BF16, kind="ExternalInput")
    x8T = nc.dram_tensor("x8T", [EPC, P, KT, C8], F8, kind="ExternalInput")
    wb = nc.dram_tensor("wb", [EPC, P, KT, DOUT], BF16, kind="ExternalInput")
    if not DEVICE_CAST_W8:
        w8 = nc.dram_tensor("w8", [EPC, P, KT, DOUT], F8,
                            kind="ExternalInput")
    yout = nc.dram_tensor("yout", [EPC, 2, max(MTB, MT8), P, DOUT], BF16,
                          kind="ExternalOutput")

    with tile.TileContext(nc) as tc:
        with ExitStack() as ctx:
            in_pool = ctx.enter_context(tc.tile_pool(name="in", bufs=1))
            out_pool = ctx.enter_context(tc.tile_pool(name="out", bufs=12))
            ps = ctx.enter_context(tc.tile_pool(name="ps", bufs=8,
                                                space="PSUM"))
            warm_pool = ctx.enter_context(tc.tile_pool(name="warm", bufs=1))
            _warmup_pe(nc, warm_pool, ps, 16, tag="ps")

            xb_ts, x8_ts, wb_ts, w8_ts = [], [], [], []
            for e in range(EPC):
                xb_ts.append(in_pool.tile([P, KT, CB], BF16, name=f"xb{e}"))
                x8_ts.append(in_pool.tile([P, KT, C8], F8, name=f"x8{e}"))
                wb_ts.append(in_pool.tile([P, KT, DOUT], BF16,
                                          name=f"wb{e}"))
                w8_ts.append(in_pool.tile([P, KT, DOUT], F8, name=f"w8{e}"))

            # --- loads: compute-phase order, round-robin over queues ----
            # k-pair granularity so arrival tracks the k-outer waves.
            qs = [nc.sync, nc.scalar, nc.gpsimd]
            qi = [0]

            def load(dst, src):
                qs[qi[0] % 3].dma_start(dst, src)
                qi[0] += 1

            # phase 1: bf16(e0) — strict k-order, equal bytes per queue
            # per k-pair (wb single-k 256KB + xb pair 256KB), so arrival
            # order across the shared-bandwidth queues matches the
            # k-outer consumption order of the first wave.
            for k in range(0, KT, 2):
                load(wb_ts[0][:, k:k + 1], wb[0, :, k:k + 1])
                load(wb_ts[0][:, k + 1:k + 2], wb[0, :, k + 1:k + 2])
                load(xb_ts[0][:, k:k + 2], xbT[0, :, k:k + 2])
            # phase 2: fp8(e0)
            load(x8_ts[0][:, 0:4], x8T[0, :, 0:4])
            load(x8_ts[0][:, 4:KT], x8T[0, :, 4:KT])
            if not DEVICE_CAST_W8:
                load(w8_ts[0][:, 0:4], w8[0, :, 0:4])
                load(w8_ts[0][:, 4:KT], w8[0, :, 4:KT])
            # phase 3: bf16(e1)
            for k in range(0, KT, 2):
                load(wb_ts[1][:, k:k + 2], wb[1, :, k:k + 2])
                load(xb_ts[1][:, k:k + 2], xbT[1, :, k:k + 2])
            # phase 4: fp8(e1)
            load(x8_ts[1][:, 0:4], x8T[1, :, 0:4])
            load(x8_ts[1][:, 4:KT], x8T[1, :, 4:KT])
            if not DEVICE_CAST_W8:
                load(w8_ts[1][:, 0:4], w8[1, :, 0:4])
                load(w8_ts[1][:, 4:KT], w8[1, :, 4:KT])

            def cast_w8(e, eng=None):
                # bf16 -> fp8 cast, k-pair granularity (|W| << 240, so a
                # plain cast cannot overflow e4m3). DVE tensor_copy or
                # ACT activation-copy, whichever engine has slack.
                for k in range(0, KT, 2):
                    if eng is nc.scalar:
                        eng.copy(w8_ts[e][:, k:k + 2],
                                 wb_ts[e][:, k:k + 2])
                    else:
                        nc.vector.tensor_copy(w8_ts[e][:, k:k + 2],
                                              wb_ts[e][:, k:k + 2])

            if DEVICE_CAST_W8:
                cast_w8(0)

            # --- compute phases ----------------------------------------
            store_ctr = [0]

            def emit_phase(e, path, waves, last_phase=False):
                f8p = (path == 1)
                x_t = (x8_ts if f8p else xb_ts)[e]
                w_t = (w8_ts if f8p else wb_ts)[e]
                MT = MT8 if f8p else MTB
                groups = [(m, n) for m in range(MT) for n in range(NT)]
                gi0 = 0
                out_tiles = {}
                nwaves = len(waves)
                for wi, wsize in enumerate(waves):
                    if isinstance(wsize, list):
                        wave = wsize
                        gi0 += len(wave)
                    else:
                        wave = groups[gi0:gi0 + wsize]
                        gi0 += wsize
                    pss = {g: ps.tile([P, NF], F32, tag="ps",
                                      name=f"ps_{e}_{path}_{g[0]}_{g[1]}")
                           for g in wave}
                    if f8p:
                        for kk in range(KT // 2):
                            for (m, n) in wave:
                                nc.tensor.matmul(
                                    pss[(m, n)][:],
                                    x_t[:, 2 * kk:2 * kk + 2,
                                        m * P:(m + 1) * P],
                                    w_t[:, 2 * kk:2 * kk + 2,
                                        n * NF:(n + 1) * NF],
                                    start=(kk == 0),
                                    stop=(kk == KT // 2 - 1),
                                    perf_mode=mybir.MatmulPerfMode.DoubleRow,
                                )
                    else:
                        for k in range(KT):
                            for (m, n) in wave:
                                nc.tensor.matmul(
                                    pss[(m, n)][:],
                                    x_t[:, k, m * P:(m + 1) * P],
                                    w_t[:, k, n * NF:(n + 1) * NF],
                                    start=(k == 0),
                                    stop=(k == KT - 1),
                                )
                    last_wave = last_phase and wi == nwaves - 1
                    fine = last_phase and wi >= nwaves - 2
                    for gi, (m, n) in enumerate(wave):
                        if m not in out_tiles:
                            out_tiles[m] = out_pool.tile(
                                [P, DOUT], BF16, tag="out",
                                name=f"out_{e}_{path}_{m}")
                        ot = out_tiles[m]
                        if gi % 2 == 1 and (f8p or last_wave):
                            # fp8 waves are only 3.46us but 4 serial DVE
                            # evictions take 2.8us — split them across
                            # ACT/DVE so the next wave's banks free in
                            # time (ACT is otherwise idle mid-run)
                            nc.scalar.copy(ot[:, n * NF:(n + 1) * NF],
                                           pss[(m, n)][:])
                        else:
                            nc.vector.tensor_copy(
                                ot[:, n * NF:(n + 1) * NF], pss[(m, n)][:])
                        if fine:
                            # drain fine-grained: store each n-half right
                            # after its eviction, spread across the two
                            # HWDGE queues (the n0 halves are early and
                            # off the critical path; the final n1 halves
                            # drain in parallel).
                            eng = nc.sync if gi % 2 == 0 else nc.scalar
                            eng.dma_start(yout[e, path, m, :,
                                               n * NF:(n + 1) * NF],
                                          ot[:, n * NF:(n + 1) * NF])
                        elif n == NT - 1:
                            eng = qs[store_ctr[0] % 3]
                            store_ctr[0] += 1
                            eng.dma_start(yout[e, path, m], ot[:])

            # bf16(e0) first wave of 6: chunk consumption (1.3us/chunk)
            # then matches the early DMA arrival rate, avoiding PE stalls
            # while the rings ramp up.
            emit_phase(0, 0, [7, 1])
            emit_phase(0, 1, [4, 4])
            if DEVICE_CAST_W8:
                cast_w8(1, eng=nc.scalar)
            emit_phase(1, 0, [4, 4])
            emit_phase(1, 1, [4, [(2, 0), (3, 0)], [(2, 1), (3, 1)]],
                       last_phase=True)
    nc.compile()
    _cache[key] = nc
    return nc


def _run(nc, in_maps):
    kw = {}
    if TRACE:
        kw["trace"] = True
    res = run_bass_kernel_spmd(nc, in_maps, list(range(NCORES)), **kw)
    if TRACE:
        LAST_EXEC_NS.append(res.exec_time_ns)
        LAST_RESULTS["last"] = res
    return res.results


def _pack(a2d, KT, C, np_dtype):
    """[DIN, n] f32 -> [P, KT, C] np_dtype, zero-padded along tokens."""
    out = np.zeros((P, KT, C), np_dtype)
    n = a2d.shape[1]
    out[:, :, :n] = (a2d.reshape(KT, P, -1).transpose(1, 0, 2)
                     .astype(np_dtype))
    return out


def kernel(x, gate_w, gate_b, expert_w, expert_b, topk):
    x = np.ascontiguousarray(np.asarray(x, dtype=np.float32))
    gate_w = np.asarray(gate_w, dtype=np.float32)
    gate_b = np.asarray(gate_b, dtype=np.float32)
    expert_w = np.asarray(expert_w, dtype=np.float32)
    expert_b = np.asarray(expert_b, dtype=np.float32)
    topk = int(topk)

    B, DIN = x.shape
    E, _, DOUT = expert_w.shape
    assert B % P == 0 and DIN % P == 0
    EPC = E // NCORES
    assert EPC * NCORES == E
    KT = DIN // P

    # ---- host: gating (softmax + top-k) in float64 ----
    logits = x.astype(np.float64) @ gate_w.astype(np.float64).T \
        + gate_b.astype(np.float64)
    order = np.argsort(-logits, axis=1, kind="stable")[:, :topk]
    z = np.exp(logits - logits.max(axis=1, keepdims=True))
    probs = z / z.sum(axis=1, keepdims=True)
    pv = np.take_along_axis(probs, order, axis=1).astype(np.float32)

    # capacity: split the mean per-expert load between the two paths
    cap = (max(P, B * topk // E) // P) * P
    CB = C8 = cap // 2

    # ---- host: routing; per expert sort by p, split bf16/fp8/host ----
    dev_b, dev_8, host_t = [], [], []
    for e in range(E):
        selmask = (order == e)
        t = np.nonzero(selmask.any(axis=1))[0]
        p = np.where(selmask[t, 0], pv[t, 0],
                     pv[t, 1] if topk > 1 else 0.0)
        o = np.argsort(-p, kind="stable")
        t, p = t[o], p[o]
        dev_b.append((t[:CB], p[:CB]))
        dev_8.append((t[CB:CB + C8], p[CB:CB + C8]))
        host_t.append((t[CB + C8:], p[CB + C8:]))

    nc = _build_mixed(CB, C8, DIN, DOUT, EPC)
    in_maps = []
    for c in range(NCORES):
        xbT = np.zeros((EPC, P, KT, CB), NPBF16)
        x8T = np.zeros((EPC, P, KT, C8), NPF8)
        wbp = np.zeros((EPC, P, KT, DOUT), NPBF16)
        w8p = None if DEVICE_CAST_W8 else \
            np.zeros((EPC, P, KT, DOUT), NPF8)
        for j in range(EPC):
            e = EPC * c + j
            tb, pb = dev_b[e]
            t8, p8 = dev_8[e]
            if len(tb):
                xbT[j] = _pack((x[tb] * pb[:, None]).T, KT, CB, NPBF16)
            if len(t8):
                x8T[j] = _pack((x[t8] * p8[:, None]).T, KT, C8, NPF8)
            wf = expert_w[e].reshape(KT, P, DOUT).transpose(1, 0, 2)
            wbp[j] = wf.astype(NPBF16)
            if w8p is not None:
                w8p[j] = wf.astype(NPF8)
        im = {"xbT": xbT, "x8T": x8T, "wb": wbp}
        if w8p is not None:
            im["w8"] = w8p
        in_maps.append(im)
    r = _run(nc, in_maps)

    # ---- host: scatter-add outputs (pure adds; both paths pre-scaled) --
    y = np.zeros((B, DOUT), np.float32)
    for c in range(NCORES):
        yo = np.asarray(r[c]["yout"])
        for j in range(EPC):
            e = EPC * c + j
            tb, _ = dev_b[e]
            t8, _ = dev_8[e]
            if len(tb):
                y[tb] += yo[j, 0].reshape(-1, DOUT)[:len(tb)] \
                    .astype(np.float32)
            if len(t8):
                y[t8] += yo[j, 1].reshape(-1, DOUT)[:len(t8)] \
                    .astype(np.float32)
    for e in range(E):
        t, p = host_t[e]
        if len(t):
            y[t] += (x[t] * p[:, None]) @ expert_w[e]
    if np.any(expert_b):
        for e in range(E):
            for (t, p) in (dev_b[e], dev_8[e], host_t[e]):
                if len(t):
                    y[t] += p[:, None] * expert_b[e][None, :]
    return y


# revision 25
# speedup vs baseline: 1.1254x; 1.1254x over previous
"""MoE routing kernel for Trainium2, 8 NeuronCores.

Strategy (expert-parallel, mixed precision, one device launch):
  Host: gating softmax + top-k in float64 (selection is exact vs the
  f32 reference since top-k margins dwarf f32 rounding noise). Per
  expert, sort its assigned tokens by gate value p (descending):
    - top CB=512 (large p)  -> bf16 path (gate-pre-scaled tokens)
    - next C8=512 (small p) -> fp8 e4m3 path (gate-pre-scaled), run
      with MatmulPerfMode.DoubleRow: 2x PE throughput. The fp8
      quantization error lands only on the low-gate half of the
      assignments, keeping total L2 error ~1.6e-2 (< 2e-2 gate).
    - remainder (~2%)       -> host f32 (standard capacity overflow)
  All tensors are packed on the host directly into SBUF layout
  [P, KT, free] so DMA descriptors have multi-KB contiguous runs per
  partition. All DMA queues share ~320 GB/s aggregate, so loads are
  emitted in compute-phase order, round-robin across the 3 queues
  (sync/scalar HWDGE + gpsimd SWDGE). The fp8 copies of the expert
  weights are produced ON DEVICE (DVE bf16->fp8 cast) instead of
  being loaded, cutting input traffic 9MB -> 7MB per core.
  Device phase order bf16(e0), fp8(e0), bf16(e1), fp8(e1): the long
  bf16 phases lead, giving the DMA stream runway; fp8 phases then
  only need the small x8 tensors plus the already-resident weights.
  fp32 PSUM; bf16 outputs (halves store traffic); warmup matmuls
  cover the PE clock ramp; small final waves keep the drain short.

PE floor: per core 128 bf16 + 64 DR matmul instrs at ~216ns issue
rate ~ 41.5us. Measured baseline (all-bf16) was ~76-81us.
"""
import numpy as np
from contextlib import ExitStack

import ml_dtypes

import concourse.mybir as mybir
from concourse import bacc, tile
from concourse.bass_utils import run_bass_kernel_spmd

NCORES = 8
P = 128
F32 = mybir.dt.float32
BF16 = mybir.dt.bfloat16
F8 = mybir.dt.float8e4
NPBF16 = ml_dtypes.bfloat16
NPF8 = ml_dtypes.float8_e4m3

# device-side bf16->fp8 weight cast (saves 2MB/core of input DMA).
# Set False to load host-quantized fp8 weights instead.
DEVICE_CAST_W8 = True

# test-harness knobs (ignored in normal use)
TRACE = False
LAST_EXEC_NS = []
LAST_RESULTS = {}

_cache = {}


def _warmup_pe(nc, pool, ps_pool, n_mm, tag="ps"):
    """Dummy bf16 matmuls on scratch data, issued at kernel start so the
    PE's HAM clock-gate ramps toward 2.4 GHz while the input DMAs
    stream in."""
    wt = pool.tile([P, 512], BF16, name="warm_sb")
    nc.vector.memset(wt[:], 1.0)
    wp = ps_pool.tile([P, 512], F32, name="warm_ps", tag=tag)
    for _ in range(n_mm):
        nc.tensor.matmul(wp[:], wt[:, :P], wt[:], start=True, stop=True)
    return wt, wp


def _build_mixed(CB, C8, DIN, DOUT, EPC):
    """Per-core expert compute, mixed bf16/fp8-DoubleRow.

    Inputs : xbT [EPC, P, KT, CB]   bf16 (pre-scaled tokens, SBUF layout)
             x8T [EPC, P, KT, C8]   f8e4 (pre-scaled tokens, SBUF layout)
             wb  [EPC, P, KT, DOUT] bf16
             (w8 [EPC, P, KT, DOUT] f8e4 -- only if not DEVICE_CAST_W8)
    Output : yout [EPC, 2, MT, P, DOUT] bf16  (path 0 = bf16, 1 = fp8)
    """
    key = ("mix", CB, C8, DIN, DOUT, EPC, DEVICE_CAST_W8)
    if key in _cache:
        return _cache[key]
    KT = DIN // P
    MTB = CB // P
    MT8 = C8 // P
    NF = 512
    assert DOUT % NF == 0 and KT % 2 == 0
    NT = DOUT // NF
    assert EPC == 2
    nc = bacc.Bacc("TRN2", target_bir_lowering=False, debug=False,
                   num_devices=NCORES)
    xbT = nc.dram_tensor("xbT", [EPC, P, KT, CB], BF16, kind="ExternalInput")
    x8T = nc.dram_tensor("x8T", [EPC, P, KT, C8], F8, kind="ExternalInput")
    wb = nc.dram_tensor("wb", [EPC, P, KT, DOUT], BF16, kind="ExternalInput")
    if not DEVICE_CAST_W8:
        w8 = nc.dram_tensor("w8", [EPC, P, KT, DOUT], F8,
                            kind="ExternalInput")
    yout = nc.dram_tensor("yout", [EPC, 2, max(MTB, MT8), P, DOUT], BF16,
                          kind="ExternalOutput")

    with tile.TileContext(nc) as tc:
        with ExitStack() as ctx:
            in_pool = ctx.enter_context(tc.tile_pool(name="in", bufs=1))
            out_pool = ctx.enter_context(tc.tile_pool(name="out", bufs=12))
            ps = ctx.enter_context(tc.tile_pool(name="ps", bufs=8,
                                                space="PSUM"))
            warm_pool = ctx.enter_context(tc.tile_pool(name="warm", bufs=1))
            _warmup_pe(nc, warm_pool, ps, 16, tag="ps")

            xb_ts, x8_ts, wb_ts, w8_ts = [], [], [], []
            for e in range(EPC):
                xb_ts.append(in_pool.tile([P, KT, CB], BF16, name=f"xb{e}"))
                x8_ts.append(in_pool.tile([P, KT, C8], F8, name=f"x8{e}"))
                wb_ts.append(in_pool.tile([P, KT, DOUT], BF16,
                                          name=f"wb{e}"))
                w8_ts.append(in_pool.tile([P, KT, DOUT], F8, name=f"w8{e}"))

            # --- loads: compute-phase order, round-robin over queues ----
            # k-pair granularity so arrival tracks the k-outer waves.
            qs = [nc.sync, nc.scalar, nc.gpsimd]
            qi = [0]

            def load(dst, src):
                qs[qi[0] % 3].dma_start(dst, src)
                qi[0] += 1

            # phase 1: bf16(e0) — strict k-order, equal bytes per queue
            # per k-pair (wb single-k 256KB + xb pair 256KB), so arrival
            # order across the shared-bandwidth queues matches the
            # k-outer consumption order of the first wave.
            for k in range(0, KT, 2):
                load(wb_ts[0][:, k:k + 1], wb[0, :, k:k + 1])
                load(wb_ts[0][:, k + 1:k + 2], wb[0, :, k + 1:k + 2])
                load(xb_ts[0][:, k:k + 2], xbT[0, :, k:k + 2])
            # phase 2: fp8(e0)
            load(x8_ts[0][:, 0:4], x8T[0, :, 0:4])
            load(x8_ts[0][:, 4:KT], x8T[0, :, 4:KT])
            if not DEVICE_CAST_W8:
                load(w8_ts[0][:, 0:4], w8[0, :, 0:4])
                load(w8_ts[0][:, 4:KT], w8[0, :, 4:KT])
            # phase 3: bf16(e1)
            for k in range(0, KT, 2):
                load(wb_ts[1][:, k:k + 2], wb[1, :, k:k + 2])
                load(xb_ts[1][:, k:k + 2], xbT[1, :, k:k + 2])
            # phase 4: fp8(e1)
            load(x8_ts[1][:, 0:4], x8T[1, :, 0:4])
            load(x8_ts[1][:, 4:KT], x8T[1, :, 4:KT])
            if not DEVICE_CAST_W8:
                load(w8_ts[1][:, 0:4], w8[1, :, 0:4])
                load(w8_ts[1][:, 4:KT], w8[1, :, 4:KT])

            def cast_w8(e, eng=None):
                # bf16 -> fp8 cast, k-pair granularity (|W| << 240, so a
                # plain cast cannot overflow e4m3). DVE tensor_copy or
                # ACT activation-copy, whichever engine has slack.
                for k in range(0, KT, 2):
                    if eng is nc.scalar:
                        eng.copy(w8_ts[e][:, k:k + 2],
                                 wb_ts[e][:, k:k + 2])
                    else:
                        nc.vector.tensor_copy(w8_ts[e][:, k:k + 2],
                                              wb_ts[e][:, k:k + 2])

            if DEVICE_CAST_W8:
                cast_w8(0)

            # --- compute phases ----------------------------------------
            store_ctr = [0]

            def emit_phase(e, path, waves, last_phase=False):
                f8p = (path == 1)
                x_t = (x8_ts if f8p else xb_ts)[e]
                w_t = (w8_ts if f8p else wb_ts)[e]
                MT = MT8 if f8p else MTB
                groups = [(m, n) for m in range(MT) for n in range(NT)]
                gi0 = 0
                out_tiles = {}
                nwaves = len(waves)
                for wi, wsize in enumerate(waves):
                    if isinstance(wsize, list):
                        wave = wsize
                        gi0 += len(wave)
                    else:
                        wave = groups[gi0:gi0 + wsize]
                        gi0 += wsize
                    pss = {g: ps.tile([P, NF], F32, tag="ps",
                                      name=f"ps_{e}_{path}_{g[0]}_{g[1]}")
                           for g in wave}
                    if f8p:
                        for kk in range(KT // 2):
                            for (m, n) in wave:
                                nc.tensor.matmul(
                                    pss[(m, n)][:],
                                    x_t[:, 2 * kk:2 * kk + 2,
                                        m * P:(m + 1) * P],
                                    w_t[:, 2 * kk:2 * kk + 2,
                                        n * NF:(n + 1) * NF],
                                    start=(kk == 0),
                                    stop=(kk == KT // 2 - 1),
                                    perf_mode=mybir.MatmulPerfMode.DoubleRow,
                                )
                    else:
                        for k in range(KT):
                            for (m, n) in wave:
                                nc.tensor.matmul(
                                    pss[(m, n)][:],
                                    x_t[:, k, m * P:(m + 1) * P],
                                    w_t[:, k, n * NF:(n + 1) * NF],
                                    start=(k == 0),
                                    stop=(k == KT - 1),
                                )
                    last_wave = last_phase and wi == nwaves - 1
                    fine = last_phase and wi >= nwaves - 3
                    for gi, (m, n) in enumerate(wave):
                        if m not in out_tiles:
                            out_tiles[m] = out_pool.tile(
                                [P, DOUT], BF16, tag="out",
                                name=f"out_{e}_{path}_{m}")
                        ot = out_tiles[m]
                        if last_wave and gi % 2 == 1:
                            nc.scalar.copy(ot[:, n * NF:(n + 1) * NF],
                                           pss[(m, n)][:])
                        else:
                            nc.vector.tensor_copy(
                                ot[:, n * NF:(n + 1) * NF], pss[(m, n)][:])
                        if fine:
                            # drain fine-grained: store each n-half right
                            # after its eviction, spread across the two
                            # HWDGE queues (the n0 halves are early and
                            # off the critical path; the final n1 halves
                            # drain in parallel).
                            eng = nc.sync if m % 2 == 0 else nc.scalar
                            eng.dma_start(yout[e, path, m, :,
                                               n * NF:(n + 1) * NF],
                                          ot[:, n * NF:(n + 1) * NF])
                        elif n == NT - 1:
                            eng = qs[store_ctr[0] % 3]
                            store_ctr[0] += 1
                            eng.dma_start(yout[e, path, m], ot[:])

            # bf16(e0) first wave of 6: chunk consumption (1.3us/chunk)
            # then matches the early DMA arrival rate, avoiding PE stalls
            # while the rings ramp up.
            emit_phase(0, 0, [7, 1])
            emit_phase(0, 1, [4, 4])
            if DEVICE_CAST_W8:
                cast_w8(1, eng=nc.scalar)
            emit_phase(1, 0, [4, 4])
            emit_phase(1, 1, [4, [(2, 0), (3, 0)], [(2, 1)], [(3, 1)]],
                       last_phase=True)
    nc.compile()
    _cache[key] = nc
    return nc


def _run(nc, in_maps):
    kw = {}
    if TRACE:
        kw["trace"] = True
    res = run_bass_kernel_spmd(nc, in_maps, list(range(NCORES)), **kw)
    if TRACE:
        LAST_EXEC_NS.append(res.exec_time_ns)
        LAST_RESULTS["last"] = res
    return res.results


def _pack(a2d, KT, C, np_dtype):
    """[DIN, n] f32 -> [P, KT, C] np_dtype, zero-padded along tokens."""
    out = np.zeros((P, KT, C), np_dtype)
    n = a2d.shape[1]
    out[:, :, :n] = (a2d.reshape(KT, P, -1).transpose(1, 0, 2)
                     .astype(np_dtype))
    return out


def kernel(x, gate_w, gate_b, expert_w, expert_b, topk):
    x = np.ascontiguousarray(np.asarray(x, dtype=np.float32))
    gate_w = np.asarray(gate_w, dtype=np.float32)
    gate_b = np.asarray(gate_b, dtype=np.float32)
    expert_w = np.asarray(expert_w, dtype=np.float32)
    expert_b = np.asarray(expert_b, dtype=np.float32)
    topk = int(topk)

    B, DIN = x.shape
    E, _, DOUT = expert_w.shape
    assert B % P == 0 and DIN % P == 0
    EPC = E // NCORES
    assert EPC * NCORES == E
    KT = DIN // P

    # ---- host: gating (softmax + top-k) in float64 ----
    logits = x.astype(np.float64) @ gate_w.astype(np.float64).T \
        + gate_b.astype(np.float64)
    order = np.argsort(-logits, axis=1, kind="stable")[:, :topk]
    z = np.exp(logits - logits.max(axis=1, keepdims=True))
    probs = z / z.sum(axis=1, keepdims=True)
    pv = np.take_along_axis(probs, order, axis=1).astype(np.float32)

    # capacity: split the mean per-expert load between the two paths
    cap = (max(P, B * topk // E) // P) * P
    CB = C8 = cap // 2

    # ---- host: routing; per expert sort by p, split bf16/fp8/host ----
    dev_b, dev_8, host_t = [], [], []
    for e in range(E):
        selmask = (order == e)
        t = np.nonzero(selmask.any(axis=1))[0]
        p = np.where(selmask[t, 0], pv[t, 0],
                     pv[t, 1] if topk > 1 else 0.0)
        o = np.argsort(-p, kind="stable")
        t, p = t[o], p[o]
        dev_b.append((t[:CB], p[:CB]))
        dev_8.append((t[CB:CB + C8], p[CB:CB + C8]))
        host_t.append((t[CB + C8:], p[CB + C8:]))

    nc = _build_mixed(CB, C8, DIN, DOUT, EPC)
    in_maps = []
    for c in range(NCORES):
        xbT = np.zeros((EPC, P, KT, CB), NPBF16)
        x8T = np.zeros((EPC, P, KT, C8), NPF8)
        wbp = np.zeros((EPC, P, KT, DOUT), NPBF16)
        w8p = None if DEVICE_CAST_W8 else \
            np.zeros((EPC, P, KT, DOUT), NPF8)
        for j in range(EPC):
            e = EPC * c + j
            tb, pb = dev_b[e]
            t8, p8 = dev_8[e]
            if len(tb):
                xbT[j] = _pack((x[tb] * pb[:, None]).T, KT, CB, NPBF16)
            if len(t8):
                x8T[j] = _pack((x[t8] * p8[:, None]).T, KT, C8, NPF8)
            wf = expert_w[e].reshape(KT, P, DOUT).transpose(1, 0, 2)
            wbp[j] = wf.astype(NPBF16)
            if w8p is not None:
                w8p[j] = wf.astype(NPF8)
        im = {"xbT": xbT, "x8T": x8T, "wb": wbp}
        if w8p is not None:
            im["w8"] = w8p
        in_maps.append(im)
    r = _run(nc, in_maps)

    # ---- host: scatter-add outputs (pure adds; both paths pre-scaled) --
    y = np.zeros((B, DOUT), np.float32)
    for c in range(NCORES):
        yo = np.asarray(r[c]["yout"])
        for j in range(EPC):
            e = EPC * c + j
            tb, _ = dev_b[e]
            t8, _ = dev_8[e]
            if len(tb):
                y[tb] += yo[j, 0].reshape(-1, DOUT)[:len(tb)] \
                    .astype(np.float32)
            if len(t8):
                y[t8] += yo[j, 1].reshape(-1, DOUT)[:len(t8)] \
                    .astype(np.float32)
    for e in range(E):
        t, p = host_t[e]
        if len(t):
            y[t] += (x[t] * p[:, None]) @ expert_w[e]
    if np.any(expert_b):
        for e in range(E):
            for (t, p) in (dev_b[e], dev_8[e], host_t[e]):
                if len(t):
                    y[t] += p[:, None] * expert_b[e][None, :]
    return y
